# revision 1
# baseline (speedup 1.0000x reference)
"""TRN2 Bass kernel for nn_Attention_5720896438407 (8-core data-parallel).

Mathematical collapse: the module computes SDPA over the *head* axis with a
single KV head (KV=1), so the softmax runs over a size-1 axis and every
attention weight is exactly 1.0.  The q path (q_a/q_norm/q_b), both rotary
embeddings, the nope/rope blend and the attention mask all cancel out, and
the module reduces to

    T  = hidden @ kv_a_w.T + kv_a_b                    # (ntok, 512)
    s  = rsqrt(mean(T^2, -1) + eps)                    # per-token RMS scale
    V  = (s*T) @ (kv_b_w[128:] * (1 + kv_norm_w)).T + kv_b_b[128:]
    Y  = V @ M.T      with  M = o_w.reshape(2048, 16, 128).sum(1)

(the attention output tiles V across all 16 heads, so o_proj sees the head
sum of its weight).  This is what the kernel computes, numerically verified
to ~3e-7 relative error against the full reference in fp64.

Distribution: pure data-parallel over the 8192 tokens — 1024 tokens per
NeuronCore, no collectives.  Per core the tokens stream through in 8 slabs
of 128 tokens; each slab's full pipeline (step-1 matmul, RMS, PE transpose,
V, Y, output DMA) starts as soon as its 0.5 MB input slab lands, so the
input stream (SP HWDGE ring) and output stream (ACT HWDGE ring) overlap for
the whole kernel.  Step-1 operands are fp16 (halves the dominant input
bytes; fp32 PSUM accumulation), the RMS statistics are computed in fp32,
the small downstream matmuls run in fp16, and Y ships as fp16 (host
casts back to fp32), halving the output stream.  End-to-end error vs the
fp32 reference is ~5e-4 relative.
"""
import sys

sys.path.insert(0, "/opt/trn_rl_repo")

import numpy as np
import concourse.bass as bass
import concourse.tile as tile
from concourse import bacc, mybir
from concourse.bass_utils import run_bass_kernel_spmd
from concourse.masks import make_identity

F32 = mybir.dt.float32
F16 = mybir.dt.float16

HID = 2048
KV = 512
D = 128
OUT = 2048
EPS = 1e-6
N_HID_CK = HID // 128   # 16
N_KV_CK = KV // 128     # 4
N_OUT_T = OUT // 512    # 4
SLAB = 128              # tokens per slab
N_CORES = 8
AF = mybir.ActivationFunctionType

_NC_CACHE = {}


def _build_nc(tok, with_ba):
    nslab = tok // SLAB
    assert tok % SLAB == 0

    nc = bacc.Bacc("TRN2", target_bir_lowering=False, debug=False,
                   num_devices=1)

    xts_d = nc.dram_tensor("xts", (nslab, 128, N_HID_CK, SLAB), F16,
                           kind="ExternalInput").ap()
    w1s_d = nc.dram_tensor("w1s", (128, N_HID_CK, KV), F16,
                           kind="ExternalInput").ap()
    wvt_d = nc.dram_tensor("wvt", (KV, D), F16, kind="ExternalInput").ap()
    mt_d = nc.dram_tensor("mt", (D, OUT), F16, kind="ExternalInput").ap()
    bv_d = nc.dram_tensor("bv", (D, 1), F32, kind="ExternalInput").ap()
    if with_ba:
        ba_d = nc.dram_tensor("bar", (1, KV), F16, kind="ExternalInput").ap()
        onesr_d = nc.dram_tensor("onesr", (1, 128), F16,
                                 kind="ExternalInput").ap()
    y_d = nc.dram_tensor("y", (tok, OUT), F16, kind="ExternalOutput").ap()

    with tile.TileContext(nc) as tc:
        with tc.tile_pool(name="consts", bufs=1) as consts, \
             tc.tile_pool(name="slabs", bufs=8) as slabs, \
             tc.tile_pool(name="work", bufs=2) as work, \
             tc.tile_pool(name="ps_t", bufs=3, space="PSUM") as ps_t, \
             tc.tile_pool(name="ps_r", bufs=2, space="PSUM") as ps_r, \
             tc.tile_pool(name="ps_v", bufs=1, space="PSUM") as ps_v, \
             tc.tile_pool(name="ps_y", bufs=2, space="PSUM") as ps_y:
            # ---- input stream on the SP ring, in priority order:
            #      W1 quarters interleaved with slab-0 quarters, then the
            #      remaining slabs (each 0.5 MB, fully contiguous thanks to
            #      the host-side swizzle) ----
            w1_s = consts.tile([128, N_HID_CK, KV], F16, tag="w1")
            sg0 = slabs.tile([128, N_HID_CK, SLAB], F16, tag="slab",
                             name="slab0")
            for h in range(4):
                nc.sync.dma_start(w1_s[:, 4 * h:4 * h + 4, :],
                                  w1s_d[:, 4 * h:4 * h + 4, :])
                nc.sync.dma_start(sg0[:, 4 * h:4 * h + 4, :],
                                  xts_d[0, :, 4 * h:4 * h + 4, :])
            sg = [sg0]
            for g in range(1, nslab):
                t = slabs.tile([128, N_HID_CK, SLAB], F16, tag="slab",
                               name=f"slab{g}")
                nc.sync.dma_start(t[:], xts_d[g])
                sg.append(t)
            # ---- small constants + all output DMAs on the ACT ring ----
            wv_s = []
            for c in range(N_KV_CK):
                t = consts.tile([128, D], F16, tag=f"wv_{c}", name=f"wv_{c}")
                nc.scalar.dma_start(t[:], wvt_d[c * 128:(c + 1) * 128, :])
                wv_s.append(t)
            mt_s = consts.tile([128, OUT], F16, tag="mt")
            nc.scalar.dma_start(mt_s[:], mt_d)
            bv_s = consts.tile([128, 1], F32, tag="bv")
            nc.scalar.dma_start(bv_s[:], bv_d)
            if with_ba:
                ba_s = consts.tile([1, KV], F16, tag="ba")
                nc.scalar.dma_start(ba_s[:], ba_d)
                onesr_s = consts.tile([1, 128], F16, tag="onesr")
                nc.scalar.dma_start(onesr_s[:], onesr_d)
            # ---- PE warm-up: junk matmuls on the (early-ready) identity
            #      keep the HAM activity monitor from throttling the PE
            #      while the first data DMAs are in flight ----
            ident = consts.tile([128, 128], F16, tag="ident")
            make_identity(nc, ident[:])
            js = consts.tile([128, 512], F16, tag="js")
            nc.gpsimd.memset(js[:], 0.0)
            junka = ps_y.tile([128, 512], F32, tag="py", name="junka")
            junkb = ps_y.tile([128, 512], F32, tag="py", name="junkb")
            for i in range(10):
                nc.tensor.matmul(junka[:] if i % 2 == 0 else junkb[:],
                                 ident[:], js[:], start=True, stop=True)
            eps_s = consts.tile([128, 1], F32, tag="eps")
            nc.vector.memset(eps_s[:], EPS)

            def step1(g):
                # T.T slab accumulation, token-major: 16 chunk matmuls,
                # fp16 operands, fp32 PSUM.
                pt = ps_t.tile([128, KV], F32, tag="pt", name=f"pt{g}")
                for ck in range(N_HID_CK):
                    nc.tensor.matmul(
                        pt[:], sg[g][:, ck, :], w1_s[:, ck, :],
                        start=(ck == 0),
                        stop=(ck == N_HID_CK - 1 and not with_ba),
                    )
                if with_ba:
                    # rank-1 row-broadcast of kv_a_b into the accumulation
                    nc.tensor.matmul(pt[:], onesr_s[:], ba_s[:],
                                     start=False, stop=True)
                return pt

            def tail(g, pt):
                t0 = g * SLAB
                # RMS statistics: Square activation with free-axis
                # accumulator gives sum(T^2) per token in one op.
                sqj = work.tile([128, KV], F32, tag="sqj")
                ssq = work.tile([128, 1], F32, tag="ssq")
                nc.scalar.activation(sqj[:], pt[:], AF.Square,
                                     accum_out=ssq[:])
                rt = work.tile([128, 1], F32, tag="rt")
                nc.scalar.activation(rt[:], ssq[:], AF.Sqrt,
                                     bias=eps_s[:], scale=1.0 / KV)
                sc = work.tile([128, 1], F32, tag="sc")
                nc.vector.reciprocal(sc[:], rt[:])
                ttn = work.tile([128, KV], F16, tag="ttn", bufs=3)
                nc.vector.tensor_scalar_mul(ttn[:], pt[:], sc[:])
                # transpose the scaled T into kv-major for step 2
                trp = ps_r.tile([128, N_KV_CK, SLAB], F16, tag="trp",
                                name=f"trp{g}")
                for c in range(N_KV_CK):
                    nc.tensor.transpose(trp[:, c, :],
                                        ttn[:, c * 128:(c + 1) * 128],
                                        ident[:])
                ttr = work.tile([128, N_KV_CK, SLAB], F16, tag="ttr", bufs=3)
                nc.vector.tensor_copy(ttr[:], trp[:])
                # step 2: V.T = Wv' @ (sT).T, bias kv_b_b on the copy
                vtp = ps_v.tile([128, SLAB], F32, tag="vtp", name=f"vtp{g}")
                for c in range(N_KV_CK):
                    nc.tensor.matmul(vtp[:], wv_s[c][:], ttr[:, c, :],
                                     start=(c == 0),
                                     stop=(c == N_KV_CK - 1))
                vts = work.tile([128, SLAB], F16, tag="vts", bufs=3)
                nc.scalar.activation(vts[:], vtp[:], AF.Identity,
                                     bias=bv_s[:], scale=1.0)
                # step 4: Y = V @ M.T, plain PSUM->SBUF copies, 1 MB DMA out
                ysb = work.tile([128, OUT], F16, tag="ysb", bufs=6)
                for n in range(N_OUT_T):
                    py = ps_y.tile([128, 512], F32, tag="py",
                                   name=f"py{g}_{n}")
                    nc.tensor.matmul(py[:], vts[:],
                                     mt_s[:, n * 512:(n + 1) * 512],
                                     start=True, stop=True)
                    ysl = ysb[:, n * 512:(n + 1) * 512]
                    if n % 2 == 0:
                        nc.vector.tensor_copy(ysl, py[:])
                    else:
                        nc.scalar.activation(ysl, py[:], AF.Copy,
                                             bias=0.0, scale=1.0)
                    if g == nslab - 1 and n == 1:
                        # final slab: overlap the first output half with the
                        # remaining matmuls/copies so only 0.25 MB trails
                        nc.scalar.dma_start(y_d[t0:t0 + SLAB, 0:1024],
                                            ysb[:, 0:1024])
                if g == nslab - 1:
                    nc.scalar.dma_start(y_d[t0:t0 + SLAB, 1024:2048],
                                        ysb[:, 1024:2048])
                else:
                    nc.scalar.dma_start(y_d[t0:t0 + SLAB, :], ysb[:])

            # 2-stage software pipeline: slab g's tail is emitted after slab
            # g+1's step-1 matmuls so the PE never waits on the RMS chain.
            prev = None
            for g in range(nslab):
                pt = step1(g)
                if prev is not None:
                    tail(*prev)
                prev = (g, pt)
            tail(*prev)

    nc.compile()
    return nc


def _host_prep(inputs):
    """Fold weights, swizzle X into fp16 token slabs, shard across cores."""
    h = np.asarray(inputs["hidden_states"], dtype=np.float32)
    b, s, hid = h.shape
    assert hid == HID
    x = np.ascontiguousarray(h.reshape(b * s, hid))
    ntok = b * s
    tok = ntok // N_CORES
    nslab = tok // SLAB

    kv_a_w = np.asarray(inputs["kv_a_w"], np.float32)
    kv_a_b = np.asarray(inputs["kv_a_b"], np.float32)
    kv_norm_w = np.asarray(inputs["kv_norm_w"], np.float32)
    kv_b_w = np.asarray(inputs["kv_b_w"], np.float32)
    kv_b_b = np.asarray(inputs["kv_b_b"], np.float32)
    o_w = np.asarray(inputs["o_w"], np.float32)

    w1s = np.ascontiguousarray(
        kv_a_w.T.reshape(N_HID_CK, 128, KV).transpose(1, 0, 2)
    ).astype(np.float16)
    wv = kv_b_w[D:2 * D] * (1.0 + kv_norm_w)[None, :]
    wvt = np.ascontiguousarray(wv.T).astype(np.float16)
    M = o_w.reshape(HID, 16, D).sum(axis=1)
    mt = np.ascontiguousarray(M.T).astype(np.float16)
    bv = np.ascontiguousarray(kv_b_b[D:2 * D].reshape(D, 1)).astype(np.float32)
    with_ba = bool(np.any(kv_a_b != 0.0))
    ba_row = np.ascontiguousarray(kv_a_b.reshape(1, KV)).astype(np.float16)
    ones_row = np.ones((1, 128), np.float16)

    in_maps = []
    for i in range(N_CORES):
        shard = x[i * tok:(i + 1) * tok]
        xts = np.ascontiguousarray(
            shard.T.reshape(N_HID_CK, 128, nslab, SLAB).transpose(2, 1, 0, 3)
        ).astype(np.float16)
        m = {"xts": xts, "w1s": w1s, "wvt": wvt, "mt": mt, "bv": bv}
        if with_ba:
            m["bar"] = ba_row
            m["onesr"] = ones_row
        in_maps.append(m)

    def gather(results):
        y = np.concatenate([r["y"] for r in results], axis=0)
        return np.ascontiguousarray(y.reshape(b, s, HID).astype(np.float32))

    return in_maps, gather, with_ba, tok


def _run(inputs, trace=False, **spmd_kwargs):
    in_maps, gather, with_ba, tok = _host_prep(inputs)
    key = (tok, with_ba)
    if key not in _NC_CACHE:
        _NC_CACHE[key] = _build_nc(tok, with_ba)
    nc = _NC_CACHE[key]
    res = run_bass_kernel_spmd(nc, in_maps, core_ids=list(range(N_CORES)),
                               trace=trace, **spmd_kwargs)
    return gather(res.results), res


def kernel(**inputs) -> np.ndarray:
    y, _ = _run(inputs, trace=False)
    return y



# revision 2
# speedup vs baseline: 1.0873x; 1.0873x over previous
"""TRN2 Bass kernel for nn_Attention_5720896438407 (8-core data-parallel).

Mathematical collapse: the module computes SDPA over the *head* axis with a
single KV head (KV=1), so every attention weight is exactly 1.0 and the whole
module reduces to (see kernel_baseline.py.bak for the derivation)

    T  = hidden @ kv_a_w.T + kv_a_b                    # (ntok, 512)
    s  = rsqrt(mean(T^2, -1) + eps)                    # per-token RMS scale
    V  = (s*T) @ Wv' + bv,   Wv' = (kv_b_w[128:256] * (1+kv_norm_w)).T
    Y  = V @ M.T,            M   = o_w.reshape(2048, 16, 128).sum(1)

This version additionally *folds the value path past the RMS norm*: since s
is a per-token scalar,

    V = s * (hidden @ Wf) + bv,   Wf = kv_a_w.T @ Wv'   (2048 x 128)

so T is only needed for the statistic s.  That lets the dominant matmul
(hidden @ kv_a_w.T, 2048-dim contraction, 512 outputs) run in fp8 DoubleRow
mode (2 MACs/cell/cycle): fp8 quantization errors are independent across the
512 columns, so the *mean* of T^2 — and hence s — keeps ~0.2% accuracy while
the matmul runs at 2x rate.  The accurate value path is the cheap rank-128
fold (hidden @ Wf, fp16) plus the output projection (V @ M.T, fp16).
Numerically verified: ~2e-3 rel-fro error vs the fp32 reference.

Distribution: pure data-parallel over the 8192 tokens — 1024 per core, no
collectives.  Per core, tokens stream in 8 slabs of 128 (two 512-token
super-slabs); the fp8 copy of X is cast on-chip by the (otherwise idle)
GpSimd engine, except the first 3 slabs which ship pre-cast from the host so
the PE can start at ~2us.  Engine layout per slab: PE does 8 DoubleRow MMs
(statistic) + 4 fp16 MMs (value, batched over the super-slab) + 4 fp16 MMs
(output); ACT does the Square+accum statistic and half the PSUM->SBUF
copies; DVE does s=1/sqrt and the other half of the copies, applying the
per-token s during the copy.  Input X/weights and output Y ride different
HWDGE rings (SP / ACT) and fully overlap compute.
"""
import sys

sys.path.insert(0, "/opt/trn_rl_repo")

import numpy as np
import ml_dtypes
import concourse.bass as bass
import concourse.tile as tile
from concourse import bacc, mybir
from concourse.bass_utils import run_bass_kernel_spmd
from concourse.masks import make_identity

F32 = mybir.dt.float32
F16 = mybir.dt.float16
F8 = mybir.dt.float8e4
DRMODE = mybir.MatmulPerfMode.DoubleRow
AF = mybir.ActivationFunctionType

HID = 2048
KV = 512
D = 128
OUT = 2048
EPS = 1e-6
WSC = 64.0                    # fp8 scale on kv_a_w (entries ~0.02)
SQ_SCALE = 1.0 / (KV * WSC * WSC)
SLAB = 128                    # tokens per slab
SS_TOK = 512                  # tokens per super-slab (value-matmul batch)
NPRE = 3                      # X8 slabs pre-cast on host
N_CORES = 8
E4 = ml_dtypes.float8_e4m3

_NC_CACHE = {}


def _build_nc(tok, with_ba):
    nss = tok // SS_TOK
    assert tok % SS_TOK == 0

    nc = bacc.Bacc("TRN2", target_bir_lowering=False, debug=False,
                   num_devices=1)

    x16_d = nc.dram_tensor("x16", (nss, 128, 4, 16, SLAB), F16,
                           kind="ExternalInput").ap()
    x8p_d = nc.dram_tensor("x8p", (128, NPRE, 8, 2, SLAB), F8,
                           kind="ExternalInput").ap()
    w18_d = nc.dram_tensor("w18", (128, 8, 2, KV), F8,
                           kind="ExternalInput").ap()
    wf_d = nc.dram_tensor("wf", (128, 16, D), F16, kind="ExternalInput").ap()
    mt_d = nc.dram_tensor("mt", (D, OUT), F16, kind="ExternalInput").ap()
    if with_ba:
        bar8_d = nc.dram_tensor("bar8", (1, KV), F8,
                                kind="ExternalInput").ap()
        cvt_d = nc.dram_tensor("cvt", (1, D), F16, kind="ExternalInput").ap()
    y_d = nc.dram_tensor("y", (tok, OUT), F16, kind="ExternalOutput").ap()

    with tile.TileContext(nc) as tc:
        with tc.tile_pool(name="consts", bufs=1) as consts, \
             tc.tile_pool(name="xs16", bufs=2) as xs16, \
             tc.tile_pool(name="xs8", bufs=2) as xs8, \
             tc.tile_pool(name="work", bufs=2) as work, \
             tc.tile_pool(name="ps_t", bufs=2, space="PSUM") as ps_t, \
             tc.tile_pool(name="ps_v", bufs=2, space="PSUM") as ps_v, \
             tc.tile_pool(name="ps_y", bufs=3, space="PSUM") as ps_y:
            # ---- SP (sync) ring: half of W18, then the X16 slab stream ----
            w18_s = consts.tile([128, 8, 2, KV], F8, tag="w18")
            nc.sync.dma_start(w18_s[:, 0:4], w18_d[:, 0:4])
            x16_t = [xs16.tile([128, 4, 16, SLAB], F16, tag="x16",
                               name=f"x16_{ss}") for ss in range(nss)]
            x8_t = [xs8.tile([128, 4, 8, 2, SLAB], F8, tag="x8",
                             name=f"x8_{ss}") for ss in range(nss)]
            for ss in range(nss):
                for j in range(4):
                    nc.sync.dma_start(x16_t[ss][:, j], x16_d[ss, :, j])
            # ---- ACT (scalar) ring: other half of W18, pre-cast X8 slabs,
            #      value/output weights, then all Y output DMAs ----
            nc.scalar.dma_start(w18_s[:, 4:8], w18_d[:, 4:8])
            for j in range(NPRE):
                nc.scalar.dma_start(x8_t[0][:, j], x8p_d[:, j])
            wf_s = consts.tile([128, 16, D], F16, tag="wf")
            nc.scalar.dma_start(wf_s[:], wf_d)
            mt_s = consts.tile([D, OUT], F16, tag="mt")
            nc.scalar.dma_start(mt_s[:], mt_d)
            if with_ba:
                bar8_s = consts.tile([1, KV], F8, tag="bar8")
                nc.scalar.dma_start(bar8_s[:], bar8_d)
                cvt_s = consts.tile([1, D], F16, tag="cvt")
                nc.scalar.dma_start(cvt_s[:], cvt_d)
                ones8_s = consts.tile([1, 128], F8, tag="ones8")
                nc.vector.memset(ones8_s[:], 1.0)
                ones16_s = consts.tile([1, SS_TOK], F16, tag="ones16")
                nc.vector.memset(ones16_s[:], 1.0)
            eps_s = consts.tile([128, 1], F32, tag="eps")
            nc.vector.memset(eps_s[:], EPS)

            # ---- PE warm-up: junk matmuls on the early-ready identity keep
            #      the HAM activity monitor from throttling the PE while the
            #      first data DMAs land ----
            ident = consts.tile([128, 128], F16, tag="ident")
            make_identity(nc, ident[:])
            js = consts.tile([128, 512], F16, tag="js")
            nc.gpsimd.memset(js[:], 0.0)
            junka = ps_y.tile([128, 512], F32, tag="py", name="junka")
            junkb = ps_y.tile([128, 512], F32, tag="py", name="junkb")
            for i in range(12):
                nc.tensor.matmul(junka[:] if i % 2 == 0 else junkb[:],
                                 ident[:], js[:], start=True, stop=True)

            def cast_slab(ss, j):
                # fp16 -> fp8 copy for the statistic path (GpSimd is idle)
                nc.gpsimd.tensor_copy(x8_t[ss][:, j], x16_t[ss][:, j])

            def norm_mm(ss, j):
                # statistic matmul: T = X @ W1 in fp8 DoubleRow, token-major
                pt = ps_t.tile([128, KV], F32, tag="pt", name=f"pt{ss}_{j}")
                for p in range(8):
                    nc.tensor.matmul(pt[:], x8_t[ss][:, j, p], w18_s[:, p],
                                     start=(p == 0),
                                     stop=(p == 7 and not with_ba),
                                     perf_mode=DRMODE)
                if with_ba:
                    # rank-1 row-broadcast of 64*kv_a_b into the accumulation
                    nc.tensor.matmul(pt[:], ones8_s[:], bar8_s[:],
                                     start=False, stop=True)
                return pt

            def stats(ss, j, pt):
                sq = work.tile([128, KV], F8, tag="sq", bufs=2)
                ssq = work.tile([128, 1], F32, tag="ssq", bufs=2)
                nc.scalar.activation(sq[:], pt[:], AF.Square,
                                     accum_out=ssq[:])
                rt = work.tile([128, 1], F32, tag="rt", bufs=2)
                nc.scalar.activation(rt[:], ssq[:], AF.Sqrt, bias=eps_s[:],
                                     scale=SQ_SCALE)
                sc = work.tile([128, 1], F32, tag="sc", bufs=8,
                               name=f"sc{ss}_{j}")
                nc.vector.reciprocal(sc[:], rt[:])
                return sc

            def value_mm(ss):
                # V.T = Wf.T @ X.T for the whole super-slab, d-major
                pv = ps_v.tile([128, SS_TOK], F32, tag="pv", name=f"pv{ss}")
                for ck in range(16):
                    nc.tensor.matmul(pv[:], wf_s[:, ck],
                                     x16_t[ss][:, :, ck, :],
                                     start=(ck == 0),
                                     stop=(ck == 15 and not with_ba))
                if with_ba:
                    # rank-1: + (kv_a_b @ Wv') per-d constant over tokens
                    nc.tensor.matmul(pv[:], cvt_s[:], ones16_s[:],
                                     start=False, stop=True)
                vts = work.tile([128, SS_TOK], F16, tag="vts", bufs=2,
                                name=f"vts{ss}")
                nc.scalar.activation(vts[:], pv[:], AF.Copy, bias=0.0,
                                     scale=1.0)
                return vts

            def step4(ss, j, vts, sc, last):
                # Y slab = s * (V.T-slab.T @ M.T); s applied during the
                # PSUM->SBUF copy (per-partition scalar on token-major out)
                t0 = (ss * 4 + j) * SLAB
                ysb = work.tile([128, OUT], F16, tag="ysb", bufs=4,
                                name=f"ysb{ss}_{j}")
                for n in range(4):
                    py = ps_y.tile([128, 512], F32, tag="py",
                                   name=f"py{ss}_{j}_{n}")
                    nc.tensor.matmul(py[:], vts[:, j * SLAB:(j + 1) * SLAB],
                                     mt_s[:, n * 512:(n + 1) * 512],
                                     start=True, stop=True)
                    ysl = ysb[:, n * 512:(n + 1) * 512]
                    if n % 2 == 0:
                        nc.vector.tensor_scalar_mul(ysl, py[:], sc[:])
                    else:
                        nc.scalar.activation(ysl, py[:], AF.Copy, bias=0.0,
                                             scale=sc[:])
                    if last and n == 1:
                        # final slab: overlap the first output half with the
                        # remaining matmuls/copies so only 0.25 MB trails
                        nc.scalar.dma_start(y_d[t0:t0 + SLAB, 0:1024],
                                            ysb[:, 0:1024])
                if last:
                    nc.scalar.dma_start(y_d[t0:t0 + SLAB, 1024:2048],
                                        ysb[:, 1024:2048])
                else:
                    nc.scalar.dma_start(y_d[t0:t0 + SLAB, :], ysb[:])

            # ---- pipeline emission (PE program order == expected readiness
            #      order so the FIFO never head-of-line blocks) ----
            scs = {}
            for ss in range(nss):
                pts = {}
                # slabs whose X8 is ready early: all but the last of ss0
                # (pre-cast covers 0..NPRE-1), none of later super-slabs
                # until their casts run; emit cast before its norm.
                for j in range(4):
                    if not (ss == 0 and j < NPRE):
                        cast_slab(ss, j)
                early = 3 if ss == 0 else 2
                for j in range(early):
                    pts[j] = norm_mm(ss, j)
                    scs[(ss, j)] = stats(ss, j, pts[j])
                vts = value_mm(ss)
                for j in range(early, 4):
                    pts[j] = norm_mm(ss, j)
                    scs[(ss, j)] = stats(ss, j, pts[j])
                for j in range(4):
                    last = (ss == nss - 1 and j == 3)
                    step4(ss, j, vts, scs[(ss, j)], last)

    nc.compile()
    return nc


def _host_prep(inputs):
    """Fold weights, swizzle X into fp16 slab layout, shard across cores."""
    h = np.asarray(inputs["hidden_states"], dtype=np.float32)
    b, s, hid = h.shape
    assert hid == HID
    x = np.ascontiguousarray(h.reshape(b * s, hid))
    ntok = b * s
    tok = ntok // N_CORES
    nss = tok // SS_TOK

    kv_a_w = np.asarray(inputs["kv_a_w"], np.float32)
    kv_a_b = np.asarray(inputs["kv_a_b"], np.float32)
    kv_norm_w = np.asarray(inputs["kv_norm_w"], np.float32)
    kv_b_w = np.asarray(inputs["kv_b_w"], np.float32)
    kv_b_b = np.asarray(inputs["kv_b_b"], np.float32)
    o_w = np.asarray(inputs["o_w"], np.float32)

    W1 = np.ascontiguousarray(kv_a_w.T)                       # (2048, 512)
    Wvp = np.ascontiguousarray(
        (kv_b_w[D:2 * D] * (1.0 + kv_norm_w)[None, :]).T)     # (512, 128)
    Wf = W1 @ Wvp                                             # (2048, 128)
    Mh = o_w.reshape(HID, 16, D).sum(axis=1)                  # (2048, 128)

    w18 = np.clip(W1 * WSC, -240, 240).reshape(16, 128, KV) \
        .transpose(1, 0, 2).reshape(128, 8, 2, KV).astype(E4)
    wfh = np.ascontiguousarray(
        Wf.reshape(16, 128, D).transpose(1, 0, 2)).astype(np.float16)
    mth = np.ascontiguousarray(Mh.T).astype(np.float16)       # (128, 2048)

    with_ba = bool(np.any(kv_a_b != 0.0))
    consts = {"w18": w18, "wf": wfh, "mt": mth}
    if with_ba:
        consts["bar8"] = np.clip(kv_a_b * WSC, -240, 240) \
            .reshape(1, KV).astype(E4)
        consts["cvt"] = (kv_a_b @ Wvp).reshape(1, D).astype(np.float16)

    in_maps = []
    for i in range(N_CORES):
        shard = x[i * tok:(i + 1) * tok]
        x16 = np.ascontiguousarray(
            shard.reshape(nss, 4, SLAB, 16, 128).transpose(0, 4, 1, 3, 2)
        ).astype(np.float16)
        x8p = np.clip(x16[0][:, 0:NPRE].astype(np.float32), -240, 240) \
            .reshape(128, NPRE, 8, 2, SLAB).astype(E4)
        m = dict(consts)
        m["x16"] = x16
        m["x8p"] = x8p
        in_maps.append(m)

    bvrow = None
    if np.any(kv_b_b[D:2 * D] != 0.0):
        bvrow = (kv_b_b[D:2 * D] @ Mh.T).astype(np.float32)   # (2048,)

    def gather(results):
        y = np.concatenate([r["y"] for r in results], axis=0) \
            .astype(np.float32)
        if bvrow is not None:
            y += bvrow[None, :]
        return np.ascontiguousarray(y.reshape(b, s, HID))

    return in_maps, gather, with_ba, tok


def _run(inputs, trace=False, **spmd_kwargs):
    in_maps, gather, with_ba, tok = _host_prep(inputs)
    key = (tok, with_ba)
    if key not in _NC_CACHE:
        _NC_CACHE[key] = _build_nc(tok, with_ba)
    nc = _NC_CACHE[key]
    res = run_bass_kernel_spmd(nc, in_maps, core_ids=list(range(N_CORES)),
                               trace=trace, **spmd_kwargs)
    return gather(res.results), res


def kernel(**inputs) -> np.ndarray:
    y, _ = _run(inputs, trace=False)
    return y


# revision 10
# speedup vs baseline: 1.0927x; 1.0050x over previous
"""TRN2 Bass kernel for nn_Attention_5720896438407 (8-core data-parallel).

Mathematical collapse: the module computes SDPA over the *head* axis with a
single KV head (KV=1), so every attention weight is exactly 1.0 and the whole
module reduces to (see kernel_baseline.py.bak for the derivation)

    T  = hidden @ kv_a_w.T + kv_a_b                    # (ntok, 512)
    s  = rsqrt(mean(T^2, -1) + eps)                    # per-token RMS scale
    V  = (s*T) @ Wv' + bv,   Wv' = (kv_b_w[128:256] * (1+kv_norm_w)).T
    Y  = V @ M.T,            M   = o_w.reshape(2048, 16, 128).sum(1)

This version additionally *folds the value path past the RMS norm*: since s
is a per-token scalar,

    V = s * (hidden @ Wf) + bv,   Wf = kv_a_w.T @ Wv'   (2048 x 128)

so T is only needed for the statistic s.  That lets the dominant matmul
(hidden @ kv_a_w.T, 2048-dim contraction, 512 outputs) run in fp8 DoubleRow
mode (2 MACs/cell/cycle): fp8 quantization errors are independent across the
512 columns, so the *mean* of T^2 — and hence s — keeps ~0.2% accuracy while
the matmul runs at 2x rate.  The accurate value path is the cheap rank-128
fold (hidden @ Wf, fp16) plus the output projection (V @ M.T, fp16).
Numerically verified: ~2e-3 rel-fro error vs the fp32 reference.

Distribution: pure data-parallel over the 8192 tokens — 1024 per core, no
collectives.  Per core, tokens stream in 8 slabs of 128 (two 512-token
super-slabs); the fp8 copy of X is cast on-chip by the (otherwise idle)
GpSimd engine, except the first 3 slabs which ship pre-cast from the host so
the PE can start at ~2us.  Engine layout per slab: PE does 8 DoubleRow MMs
(statistic) + 4 fp16 MMs (value, batched over the super-slab) + 4 fp16 MMs
(output); ACT does the Square+accum statistic and half the PSUM->SBUF
copies; DVE does s=1/sqrt and the other half of the copies, applying the
per-token s during the copy.  Input X/weights and output Y ride different
HWDGE rings (SP / ACT) and fully overlap compute.
"""
import sys

sys.path.insert(0, "/opt/trn_rl_repo")

import numpy as np
import ml_dtypes
import concourse.bass as bass
import concourse.tile as tile
from concourse import bacc, mybir
from concourse.bass_utils import run_bass_kernel_spmd
from concourse.masks import make_identity

F32 = mybir.dt.float32
F16 = mybir.dt.float16
F8 = mybir.dt.float8e4
DRMODE = mybir.MatmulPerfMode.DoubleRow
AF = mybir.ActivationFunctionType

HID = 2048
KV = 512
D = 128
OUT = 2048
EPS = 1e-6
WSC = 64.0                    # fp8 scale on kv_a_w (entries ~0.02)
SQ_SCALE = 1.0 / (KV * WSC * WSC)
SLAB = 128                    # tokens per slab
SS_TOK = 512                  # tokens per super-slab (value-matmul batch)
NPRE = 3                      # X8 slabs pre-cast on host
N_CORES = 8
E4 = ml_dtypes.float8_e4m3

_NC_CACHE = {}


def _build_nc(tok, with_ba):
    nss = tok // SS_TOK
    assert tok % SS_TOK == 0 and nss == 2

    nc = bacc.Bacc("TRN2", target_bir_lowering=False, debug=False,
                   num_devices=1)

    x16_d = nc.dram_tensor("x16", (nss, 128, 4, 16, SLAB), F16,
                           kind="ExternalInput").ap()
    x8p_d = nc.dram_tensor("x8p", (nss, 128, 4, 8, 2, SLAB), F8,
                           kind="ExternalInput").ap()
    w18_d = nc.dram_tensor("w18", (128, 8, 2, KV), F8,
                           kind="ExternalInput").ap()
    wf_d = nc.dram_tensor("wf", (128, 16, D), F16, kind="ExternalInput").ap()
    mt_d = nc.dram_tensor("mt", (D, OUT), F16, kind="ExternalInput").ap()
    if with_ba:
        bar8_d = nc.dram_tensor("bar8", (1, KV), F8,
                                kind="ExternalInput").ap()
        cvt_d = nc.dram_tensor("cvt", (1, D), F16, kind="ExternalInput").ap()
    y_d = nc.dram_tensor("y", (tok, OUT), F16, kind="ExternalOutput").ap()

    with tile.TileContext(nc) as tc:
        with tc.tile_pool(name="consts", bufs=1) as consts, \
             tc.tile_pool(name="xs16", bufs=2) as xs16, \
             tc.tile_pool(name="xs8", bufs=2) as xs8, \
             tc.tile_pool(name="work", bufs=2) as work, \
             tc.tile_pool(name="ps_t", bufs=2, space="PSUM") as ps_t, \
             tc.tile_pool(name="ps_v", bufs=2, space="PSUM") as ps_v, \
             tc.tile_pool(name="ps_y", bufs=3, space="PSUM") as ps_y:
            # ---- SP (sync) ring: the X16 slab stream, front-loaded so the
            #      value matmuls unblock as early as possible; the last two
            #      X16 slabs ride the ACT ring so both rings finish the
            #      input stream together ----
            w18_s = consts.tile([128, 8, 2, KV], F8, tag="w18")
            x16_t = [xs16.tile([128, 4, 16, SLAB], F16, tag="x16",
                               name=f"x16_{ss}") for ss in range(nss)]
            x8_t = [xs8.tile([128, 4, 8, 2, SLAB], F8, tag="x8",
                             name=f"x8_{ss}") for ss in range(nss)]
            for j in range(4):
                nc.sync.dma_start(x16_t[0][:, j], x16_d[0, :, j])
            nc.sync.dma_start(x16_t[1][:, 0], x16_d[1, :, 0])
            nc.sync.dma_start(x16_t[1][:, 1], x16_d[1, :, 1])
            # ---- ACT (scalar) ring: W18 quarters + X8 slabs (statistic
            #      path operands), the two trailing X16 slabs, weights ----
            nc.scalar.dma_start(x8_t[0][:, 0], x8p_d[0, :, 0])
            for p in range(0, 8, 2):
                nc.scalar.dma_start(w18_s[:, p:p + 2], w18_d[:, p:p + 2])
            for j in range(1, 4):
                nc.scalar.dma_start(x8_t[0][:, j], x8p_d[0, :, j])
            nc.scalar.dma_start(x16_t[1][:, 2], x16_d[1, :, 2])
            nc.scalar.dma_start(x16_t[1][:, 3], x16_d[1, :, 3])
            wf_s = consts.tile([128, 16, D], F16, tag="wf")
            nc.scalar.dma_start(wf_s[:], wf_d)
            for j in range(4):
                nc.scalar.dma_start(x8_t[1][:, j], x8p_d[1, :, j])
            mt_s = consts.tile([D, OUT], F16, tag="mt")
            nc.scalar.dma_start(mt_s[:], mt_d)
            if with_ba:
                bar8_s = consts.tile([1, KV], F8, tag="bar8")
                nc.scalar.dma_start(bar8_s[:], bar8_d)
                cvt_s = consts.tile([1, D], F16, tag="cvt")
                nc.scalar.dma_start(cvt_s[:], cvt_d)
                ones8_s = consts.tile([1, 128], F8, tag="ones8")
                nc.vector.memset(ones8_s[:], 1.0)
                ones16_s = consts.tile([1, SS_TOK], F16, tag="ones16")
                nc.vector.memset(ones16_s[:], 1.0)
            eps_s = consts.tile([128, 1], F32, tag="eps")
            nc.vector.memset(eps_s[:], EPS)

            # ---- PE warm-up: junk matmuls on an early-ready zero tile keep
            #      the HAM activity monitor from throttling the PE while the
            #      first data DMAs land ----
            js = consts.tile([128, 512], F16, tag="js")
            nc.gpsimd.memset(js[:], 0.0)
            junka = ps_y.tile([128, 512], F32, tag="py", name="junka")
            junkb = ps_y.tile([128, 512], F32, tag="py", name="junkb")
            for i in range(8):
                nc.tensor.matmul(junka[:] if i % 2 == 0 else junkb[:],
                                 js[:, 0:128], js[:], start=True, stop=True)

            def norm_mm(ss, j):
                # statistic matmul: T = X @ W1 in fp8 DoubleRow, token-major
                pt = ps_t.tile([128, KV], F32, tag="pt", name=f"pt{ss}_{j}")
                for p in range(8):
                    nc.tensor.matmul(pt[:], x8_t[ss][:, j, p], w18_s[:, p],
                                     start=(p == 0),
                                     stop=(p == 7 and not with_ba),
                                     perf_mode=DRMODE)
                if with_ba:
                    # rank-1 row-broadcast of 64*kv_a_b into the accumulation
                    nc.tensor.matmul(pt[:], ones8_s[:], bar8_s[:],
                                     start=False, stop=True)
                return pt

            def stats(ss, j, pt):
                sq = work.tile([128, KV], F8, tag="sq", bufs=2)
                ssq = work.tile([128, 1], F32, tag="ssq", bufs=2)
                nc.scalar.activation(sq[:], pt[:], AF.Square,
                                     accum_out=ssq[:])
                rt = work.tile([128, 1], F32, tag="rt", bufs=2)
                nc.scalar.activation(rt[:], ssq[:], AF.Sqrt, bias=eps_s[:],
                                     scale=SQ_SCALE)
                sc = work.tile([128, 1], F32, tag="sc", bufs=8,
                               name=f"sc{ss}_{j}")
                nc.vector.reciprocal(sc[:], rt[:])
                return sc

            def value_mm(ss):
                # V.T = Wf.T @ X.T for the whole super-slab, d-major
                pv = ps_v.tile([128, SS_TOK], F32, tag="pv", name=f"pv{ss}")
                for ck in range(16):
                    nc.tensor.matmul(pv[:], wf_s[:, ck],
                                     x16_t[ss][:, :, ck, :],
                                     start=(ck == 0),
                                     stop=(ck == 15 and not with_ba))
                if with_ba:
                    # rank-1: + (kv_a_b @ Wv') per-d constant over tokens
                    nc.tensor.matmul(pv[:], cvt_s[:], ones16_s[:],
                                     start=False, stop=True)
                vts = work.tile([128, SS_TOK], F16, tag="vts", bufs=2,
                                name=f"vts{ss}")
                nc.scalar.activation(vts[:], pv[:], AF.Copy, bias=0.0,
                                     scale=1.0)
                return vts

            def step4(ss, j, vts, sc, last):
                # Y slab = s * (V.T-slab.T @ M.T); s applied during the
                # PSUM->SBUF copy (per-partition scalar on token-major out)
                t0 = (ss * 4 + j) * SLAB
                ysb = work.tile([128, OUT], F16, tag="ysb", bufs=4,
                                name=f"ysb{ss}_{j}")
                for n in range(4):
                    py = ps_y.tile([128, 512], F32, tag="py",
                                   name=f"py{ss}_{j}_{n}")
                    nc.tensor.matmul(py[:], vts[:, j * SLAB:(j + 1) * SLAB],
                                     mt_s[:, n * 512:(n + 1) * 512],
                                     start=True, stop=True)
                    ysl = ysb[:, n * 512:(n + 1) * 512]
                    if n % 2 == 0:
                        nc.vector.tensor_scalar_mul(ysl, py[:], sc[:])
                    else:
                        nc.scalar.activation(ysl, py[:], AF.Copy, bias=0.0,
                                             scale=sc[:])
                    if last and n == 1:
                        # final slab: overlap the first output half with the
                        # remaining matmuls/copies so only 0.25 MB trails
                        nc.scalar.dma_start(y_d[t0:t0 + SLAB, 0:1024],
                                            ysb[:, 0:1024])
                # alternate output DMAs across the two HWDGE rings
                eng = nc.scalar if (ss * 4 + j) % 2 == 0 else nc.sync
                if last:
                    nc.scalar.dma_start(y_d[t0:t0 + SLAB, 1024:2048],
                                        ysb[:, 1024:2048])
                else:
                    eng.dma_start(y_d[t0:t0 + SLAB, :], ysb[:])

            # ---- pipeline emission (PE program order == expected readiness
            #      order so the FIFO never head-of-line blocks) ----
            scs = {}
            for j in range(4):
                scs[(0, j)] = stats(0, j, norm_mm(0, j))
            vts0 = value_mm(0)
            # first norm of ss1 is ready before ss0's vts copy lands
            scs[(1, 0)] = stats(1, 0, norm_mm(1, 0))
            for j in range(4):
                step4(0, j, vts0, scs[(0, j)], False)
            for j in range(1, 4):
                scs[(1, j)] = stats(1, j, norm_mm(1, j))
            vts1 = value_mm(1)
            for j in range(4):
                step4(1, j, vts1, scs[(1, j)], j == 3)

    nc.compile()
    return nc


def _host_prep(inputs):
    """Fold weights, swizzle X into fp16 slab layout, shard across cores."""
    h = np.asarray(inputs["hidden_states"], dtype=np.float32)
    b, s, hid = h.shape
    assert hid == HID
    x = np.ascontiguousarray(h.reshape(b * s, hid))
    ntok = b * s
    tok = ntok // N_CORES
    nss = tok // SS_TOK

    kv_a_w = np.asarray(inputs["kv_a_w"], np.float32)
    kv_a_b = np.asarray(inputs["kv_a_b"], np.float32)
    kv_norm_w = np.asarray(inputs["kv_norm_w"], np.float32)
    kv_b_w = np.asarray(inputs["kv_b_w"], np.float32)
    kv_b_b = np.asarray(inputs["kv_b_b"], np.float32)
    o_w = np.asarray(inputs["o_w"], np.float32)

    W1 = np.ascontiguousarray(kv_a_w.T)                       # (2048, 512)
    Wvp = np.ascontiguousarray(
        (kv_b_w[D:2 * D] * (1.0 + kv_norm_w)[None, :]).T)     # (512, 128)
    Wf = W1 @ Wvp                                             # (2048, 128)
    Mh = o_w.reshape(HID, 16, D).sum(axis=1)                  # (2048, 128)

    w18 = np.clip(W1 * WSC, -240, 240).reshape(16, 128, KV) \
        .transpose(1, 0, 2).reshape(128, 8, 2, KV).astype(E4)
    wfh = np.ascontiguousarray(
        Wf.reshape(16, 128, D).transpose(1, 0, 2)).astype(np.float16)
    mth = np.ascontiguousarray(Mh.T).astype(np.float16)       # (128, 2048)

    with_ba = bool(np.any(kv_a_b != 0.0))
    consts = {"w18": w18, "wf": wfh, "mt": mth}
    if with_ba:
        consts["bar8"] = np.clip(kv_a_b * WSC, -240, 240) \
            .reshape(1, KV).astype(E4)
        consts["cvt"] = (kv_a_b @ Wvp).reshape(1, D).astype(np.float16)

    in_maps = []
    for i in range(N_CORES):
        shard = x[i * tok:(i + 1) * tok]
        x16 = np.ascontiguousarray(
            shard.reshape(nss, 4, SLAB, 16, 128).transpose(0, 4, 1, 3, 2)
        ).astype(np.float16)
        x8p = np.clip(x16.astype(np.float32), -240, 240) \
            .reshape(nss, 128, 4, 8, 2, SLAB).astype(E4)
        m = dict(consts)
        m["x16"] = x16
        m["x8p"] = x8p
        in_maps.append(m)

    bvrow = None
    if np.any(kv_b_b[D:2 * D] != 0.0):
        bvrow = (kv_b_b[D:2 * D] @ Mh.T).astype(np.float32)   # (2048,)

    def gather(results):
        y = np.concatenate([r["y"] for r in results], axis=0) \
            .astype(np.float32)
        if bvrow is not None:
            y += bvrow[None, :]
        return np.ascontiguousarray(y.reshape(b, s, HID))

    return in_maps, gather, with_ba, tok


def _run(inputs, trace=False, **spmd_kwargs):
    in_maps, gather, with_ba, tok = _host_prep(inputs)
    key = (tok, with_ba)
    if key not in _NC_CACHE:
        _NC_CACHE[key] = _build_nc(tok, with_ba)
    nc = _NC_CACHE[key]
    res = run_bass_kernel_spmd(nc, in_maps, core_ids=list(range(N_CORES)),
                               trace=trace, **spmd_kwargs)
    return gather(res.results), res


def kernel(**inputs) -> np.ndarray:
    y, _ = _run(inputs, trace=False)
    return y


# revision 14
# speedup vs baseline: 1.2457x; 1.1400x over previous
"""TRN2 Bass kernel for nn_Attention_5720896438407 (8-core data-parallel).

Mathematical collapse: the module computes SDPA over the *head* axis with a
single KV head (KV=1), so every attention weight is exactly 1.0 and the whole
module reduces to (see kernel_baseline.py.bak for the derivation)

    T  = hidden @ kv_a_w.T + kv_a_b                    # (ntok, 512)
    s  = rsqrt(mean(T^2, -1) + eps)                    # per-token RMS scale
    V  = (s*T) @ Wv' + bv,   Wv' = (kv_b_w[128:256] * (1+kv_norm_w)).T
    Y  = V @ M.T,            M   = o_w.reshape(2048, 16, 128).sum(1)

This version additionally *folds the value path past the RMS norm*: since s
is a per-token scalar,

    V = s * (hidden @ Wf) + bv,   Wf = kv_a_w.T @ Wv'   (2048 x 128)

so T is only needed for the statistic s.  That lets the dominant matmul
(hidden @ kv_a_w.T, 2048-dim contraction, 512 outputs) run in fp8 DoubleRow
mode (2 MACs/cell/cycle): fp8 quantization errors are independent across the
512 columns, so the *mean* of T^2 — and hence s — keeps ~0.2% accuracy while
the matmul runs at 2x rate.  The accurate value path is the cheap rank-128
fold (hidden @ Wf, fp16) plus the output projection (V @ M.T, fp16).
Numerically verified: ~2e-3 rel-fro error vs the fp32 reference.

Distribution: pure data-parallel over the 8192 tokens — 1024 per core, no
collectives.  Per core, tokens stream in 8 slabs of 128 (two 512-token
super-slabs); the fp8 copy of X is cast on-chip by the (otherwise idle)
GpSimd engine, except the first 3 slabs which ship pre-cast from the host so
the PE can start at ~2us.  Engine layout per slab: PE does 8 DoubleRow MMs
(statistic) + 4 fp16 MMs (value, batched over the super-slab) + 4 fp16 MMs
(output); ACT does the Square+accum statistic and half the PSUM->SBUF
copies; DVE does s=1/sqrt and the other half of the copies, applying the
per-token s during the copy.  Input X/weights and output Y ride different
HWDGE rings (SP / ACT) and fully overlap compute.
"""
import sys

sys.path.insert(0, "/opt/trn_rl_repo")

import numpy as np
import ml_dtypes
import concourse.bass as bass
import concourse.tile as tile
from concourse import bacc, mybir
from concourse.bass_utils import run_bass_kernel_spmd
from concourse.masks import make_identity

F32 = mybir.dt.float32
F16 = mybir.dt.float16
F8 = mybir.dt.float8e4
DRMODE = mybir.MatmulPerfMode.DoubleRow
AF = mybir.ActivationFunctionType

HID = 2048
KV = 512
D = 128
OUT = 2048
EPS = 1e-6
WSC = 64.0                    # fp8 scale on kv_a_w (entries ~0.02)
SQ_SCALE = 1.0 / (KV * WSC * WSC)
SLAB = 128                    # tokens per slab
SS_TOK = 512                  # tokens per super-slab (value-matmul batch)
NPRE = 3                      # X8 slabs pre-cast on host
N_CORES = 8
E4 = ml_dtypes.float8_e4m3

_NC_CACHE = {}


def _build_nc(tok, with_ba):
    nss = tok // SS_TOK
    assert tok % SS_TOK == 0 and nss == 2

    nc = bacc.Bacc("TRN2", target_bir_lowering=False, debug=False,
                   num_devices=1)

    x16_d = nc.dram_tensor("x16", (nss, 128, 4, 16, SLAB), F16,
                           kind="ExternalInput").ap()
    x8p_d = nc.dram_tensor("x8p", (nss, 128, 4, 8, 2, SLAB), F8,
                           kind="ExternalInput").ap()
    w18_d = nc.dram_tensor("w18", (128, 8, 2, KV), F8,
                           kind="ExternalInput").ap()
    wf_d = nc.dram_tensor("wf", (128, 16, D), F16, kind="ExternalInput").ap()
    mt_d = nc.dram_tensor("mt", (D, OUT), F16, kind="ExternalInput").ap()
    if with_ba:
        bar8_d = nc.dram_tensor("bar8", (1, KV), F8,
                                kind="ExternalInput").ap()
        cvt_d = nc.dram_tensor("cvt", (1, D), F16, kind="ExternalInput").ap()
    y_d = nc.dram_tensor("y", (tok, OUT), F16, kind="ExternalOutput").ap()

    with tile.TileContext(nc) as tc:
        with tc.tile_pool(name="consts", bufs=1) as consts, \
             tc.tile_pool(name="xs16", bufs=2) as xs16, \
             tc.tile_pool(name="xs8", bufs=2) as xs8, \
             tc.tile_pool(name="work", bufs=2) as work, \
             tc.tile_pool(name="ps_t", bufs=2, space="PSUM") as ps_t, \
             tc.tile_pool(name="ps_v", bufs=2, space="PSUM") as ps_v, \
             tc.tile_pool(name="ps_y", bufs=3, space="PSUM") as ps_y:
            # ---- DMA schedule.  Ring rates are ~210 GB/s each (~0.21
            #      MB/us); SP's first packet lands ~1.5 us before ACT's.
            #      First bytes on each ring are the statistic-path operands
            #      (w18 quarters + x8 slab 0) so the PE can start at ~11 us;
            #      the X16 stream follows on SP; ACT carries the remaining
            #      x8 slabs and the small weights. ----
            w18_s = consts.tile([128, 8, 2, KV], F8, tag="w18")
            x16_t = [xs16.tile([128, 4, 16, SLAB], F16, tag="x16",
                               name=f"x16_{ss}") for ss in range(nss)]
            x8_t = [xs8.tile([128, 4, 8, 2, SLAB], F8, tag="x8",
                             name=f"x8_{ss}") for ss in range(nss)]
            # SP ring
            nc.sync.dma_start(x8_t[0][:, 0], x8p_d[0, :, 0])
            nc.sync.dma_start(w18_s[:, 0:2], w18_d[:, 0:2])
            for j in range(4):
                nc.sync.dma_start(x16_t[0][:, j], x16_d[0, :, j])
            for j in range(4):
                nc.sync.dma_start(x16_t[1][:, j], x16_d[1, :, j])
            # ACT ring
            for p in range(2, 8, 2):
                nc.scalar.dma_start(w18_s[:, p:p + 2], w18_d[:, p:p + 2])
            for j in range(1, 4):
                nc.scalar.dma_start(x8_t[0][:, j], x8p_d[0, :, j])
            wf_s = consts.tile([128, 16, D], F16, tag="wf")
            nc.scalar.dma_start(wf_s[:], wf_d)
            mt_s = consts.tile([D, OUT], F16, tag="mt")
            nc.scalar.dma_start(mt_s[:], mt_d)
            for j in range(4):
                nc.scalar.dma_start(x8_t[1][:, j], x8p_d[1, :, j])
            if with_ba:
                bar8_s = consts.tile([1, KV], F8, tag="bar8")
                nc.scalar.dma_start(bar8_s[:], bar8_d)
                cvt_s = consts.tile([1, D], F16, tag="cvt")
                nc.scalar.dma_start(cvt_s[:], cvt_d)
                ones8_s = consts.tile([1, 128], F8, tag="ones8")
                nc.vector.memset(ones8_s[:], 1.0)
                ones16_s = consts.tile([1, SS_TOK], F16, tag="ones16")
                nc.vector.memset(ones16_s[:], 1.0)
            eps_s = consts.tile([128, 1], F32, tag="eps")
            nc.vector.memset(eps_s[:], EPS)

            # ---- PE warm-up / keep-alive: junk matmuls on an early-ready
            #      zero tile keep the HAM activity monitor from throttling
            #      the PE while data DMAs are in flight ----
            js = consts.tile([128, 512], F16, tag="js")
            nc.gpsimd.memset(js[:], 0.0)
            junkt = [ps_y.tile([128, 512], F32, tag="py", name=f"junk{i}")
                     for i in range(2)]
            jn = [0]

            def junk(n):
                for _ in range(n):
                    i = jn[0] = jn[0] + 1
                    nc.tensor.matmul(junkt[i % 2][:], js[:, 0:128], js[:],
                                     start=True, stop=True)

            junk(8)

            def norm_mm(ss, j, filler=False):
                # statistic matmul: T = X @ W1 in fp8 DoubleRow, token-major
                pt = ps_t.tile([128, KV], F32, tag="pt", name=f"pt{ss}_{j}")
                for p in range(8):
                    nc.tensor.matmul(pt[:], x8_t[ss][:, j, p], w18_s[:, p],
                                     start=(p == 0),
                                     stop=(p == 7 and not with_ba),
                                     perf_mode=DRMODE)
                    if filler and p % 2 == 1 and p < 7:
                        # first slab is paced by the w18 quarter DMAs; keep
                        # the PE clock warm between pair arrivals
                        junk(1)
                if with_ba:
                    # rank-1 row-broadcast of 64*kv_a_b into the accumulation
                    nc.tensor.matmul(pt[:], ones8_s[:], bar8_s[:],
                                     start=False, stop=True)
                return pt

            def stats(ss, j, pt):
                sq = work.tile([128, KV], F8, tag="sq", bufs=2)
                ssq = work.tile([128, 1], F32, tag="ssq", bufs=2)
                nc.scalar.activation(sq[:], pt[:], AF.Square,
                                     accum_out=ssq[:])
                rt = work.tile([128, 1], F32, tag="rt", bufs=2)
                nc.scalar.activation(rt[:], ssq[:], AF.Sqrt, bias=eps_s[:],
                                     scale=SQ_SCALE)
                sc = work.tile([128, 1], F32, tag="sc", bufs=8,
                               name=f"sc{ss}_{j}")
                nc.vector.reciprocal(sc[:], rt[:])
                return sc

            def value_mm(ss, h):
                # V.T = Wf.T @ X.T for one half-super-slab (2 slabs,
                # N=256), d-major; halves unblock as their X16 slabs land
                pv = ps_v.tile([128, 2 * SLAB], F32, tag="pv", bufs=3,
                               name=f"pv{ss}_{h}")
                for ck in range(16):
                    nc.tensor.matmul(pv[:], wf_s[:, ck],
                                     x16_t[ss][:, 2 * h:2 * h + 2, ck, :],
                                     start=(ck == 0),
                                     stop=(ck == 15 and not with_ba))
                if with_ba:
                    # rank-1: + (kv_a_b @ Wv') per-d constant over tokens
                    nc.tensor.matmul(pv[:], cvt_s[:],
                                     ones16_s[:, 0:2 * SLAB],
                                     start=False, stop=True)
                vts = work.tile([128, 2 * SLAB], F16, tag="vts", bufs=4,
                                name=f"vts{ss}_{h}")
                nc.scalar.activation(vts[:], pv[:], AF.Copy, bias=0.0,
                                     scale=1.0)
                return vts

            def step4(ss, j, vts, sc, last):
                # Y slab = s * (V.T-slab.T @ M.T); s applied during the
                # PSUM->SBUF copy (per-partition scalar on token-major out)
                t0 = (ss * 4 + j) * SLAB
                jh = (j % 2) * SLAB
                ysb = work.tile([128, OUT], F16, tag="ysb", bufs=4,
                                name=f"ysb{ss}_{j}")
                for n in range(4):
                    py = ps_y.tile([128, 512], F32, tag="py",
                                   name=f"py{ss}_{j}_{n}")
                    nc.tensor.matmul(py[:], vts[:, jh:jh + SLAB],
                                     mt_s[:, n * 512:(n + 1) * 512],
                                     start=True, stop=True)
                    ysl = ysb[:, n * 512:(n + 1) * 512]
                    if n % 2 == 0:
                        nc.vector.tensor_scalar_mul(ysl, py[:], sc[:])
                    else:
                        nc.scalar.activation(ysl, py[:], AF.Copy, bias=0.0,
                                             scale=sc[:])
                    if last and n == 1:
                        # final slab: overlap the first output half with the
                        # remaining matmuls/copies so only 0.25 MB trails
                        nc.scalar.dma_start(y_d[t0:t0 + SLAB, 0:1024],
                                            ysb[:, 0:1024])
                # alternate output DMAs across the two HWDGE rings
                eng = nc.scalar if (ss * 4 + j) % 2 == 0 else nc.sync
                if last:
                    nc.scalar.dma_start(y_d[t0:t0 + SLAB, 1024:2048],
                                        ysb[:, 1024:2048])
                else:
                    eng.dma_start(y_d[t0:t0 + SLAB, :], ysb[:])

            # ---- pipeline emission (PE program order == expected readiness
            #      order so the FIFO never head-of-line blocks) ----
            scs = {}
            scs[(0, 0)] = stats(0, 0, norm_mm(0, 0, filler=True))
            junk(2)
            scs[(0, 1)] = stats(0, 1, norm_mm(0, 1))
            scs[(0, 2)] = stats(0, 2, norm_mm(0, 2))
            v0a = value_mm(0, 0)
            scs[(0, 3)] = stats(0, 3, norm_mm(0, 3))
            v0b = value_mm(0, 1)
            step4(0, 0, v0a, scs[(0, 0)], False)
            step4(0, 1, v0a, scs[(0, 1)], False)
            scs[(1, 0)] = stats(1, 0, norm_mm(1, 0))
            step4(0, 2, v0b, scs[(0, 2)], False)
            step4(0, 3, v0b, scs[(0, 3)], False)
            scs[(1, 1)] = stats(1, 1, norm_mm(1, 1))
            scs[(1, 2)] = stats(1, 2, norm_mm(1, 2))
            v1a = value_mm(1, 0)
            scs[(1, 3)] = stats(1, 3, norm_mm(1, 3))
            v1b = value_mm(1, 1)
            step4(1, 0, v1a, scs[(1, 0)], False)
            step4(1, 1, v1a, scs[(1, 1)], False)
            step4(1, 2, v1b, scs[(1, 2)], False)
            step4(1, 3, v1b, scs[(1, 3)], True)

    nc.compile()
    return nc


def _host_prep(inputs):
    """Fold weights, swizzle X into fp16 slab layout, shard across cores."""
    h = np.asarray(inputs["hidden_states"], dtype=np.float32)
    b, s, hid = h.shape
    assert hid == HID
    x = np.ascontiguousarray(h.reshape(b * s, hid))
    ntok = b * s
    tok = ntok // N_CORES
    nss = tok // SS_TOK

    kv_a_w = np.asarray(inputs["kv_a_w"], np.float32)
    kv_a_b = np.asarray(inputs["kv_a_b"], np.float32)
    kv_norm_w = np.asarray(inputs["kv_norm_w"], np.float32)
    kv_b_w = np.asarray(inputs["kv_b_w"], np.float32)
    kv_b_b = np.asarray(inputs["kv_b_b"], np.float32)
    o_w = np.asarray(inputs["o_w"], np.float32)

    W1 = np.ascontiguousarray(kv_a_w.T)                       # (2048, 512)
    Wvp = np.ascontiguousarray(
        (kv_b_w[D:2 * D] * (1.0 + kv_norm_w)[None, :]).T)     # (512, 128)
    Wf = W1 @ Wvp                                             # (2048, 128)
    Mh = o_w.reshape(HID, 16, D).sum(axis=1)                  # (2048, 128)

    w18 = np.clip(W1 * WSC, -240, 240).reshape(16, 128, KV) \
        .transpose(1, 0, 2).reshape(128, 8, 2, KV).astype(E4)
    wfh = np.ascontiguousarray(
        Wf.reshape(16, 128, D).transpose(1, 0, 2)).astype(np.float16)
    mth = np.ascontiguousarray(Mh.T).astype(np.float16)       # (128, 2048)

    with_ba = bool(np.any(kv_a_b != 0.0))
    consts = {"w18": w18, "wf": wfh, "mt": mth}
    if with_ba:
        consts["bar8"] = np.clip(kv_a_b * WSC, -240, 240) \
            .reshape(1, KV).astype(E4)
        consts["cvt"] = (kv_a_b @ Wvp).reshape(1, D).astype(np.float16)

    in_maps = []
    for i in range(N_CORES):
        shard = x[i * tok:(i + 1) * tok]
        x16 = np.ascontiguousarray(
            shard.reshape(nss, 4, SLAB, 16, 128).transpose(0, 4, 1, 3, 2)
        ).astype(np.float16)
        x8p = np.clip(x16.astype(np.float32), -240, 240) \
            .reshape(nss, 128, 4, 8, 2, SLAB).astype(E4)
        m = dict(consts)
        m["x16"] = x16
        m["x8p"] = x8p
        in_maps.append(m)

    bvrow = None
    if np.any(kv_b_b[D:2 * D] != 0.0):
        bvrow = (kv_b_b[D:2 * D] @ Mh.T).astype(np.float32)   # (2048,)

    def gather(results):
        y = np.concatenate([r["y"] for r in results], axis=0) \
            .astype(np.float32)
        if bvrow is not None:
            y += bvrow[None, :]
        return np.ascontiguousarray(y.reshape(b, s, HID))

    return in_maps, gather, with_ba, tok


def _run(inputs, trace=False, **spmd_kwargs):
    in_maps, gather, with_ba, tok = _host_prep(inputs)
    key = (tok, with_ba)
    if key not in _NC_CACHE:
        _NC_CACHE[key] = _build_nc(tok, with_ba)
    nc = _NC_CACHE[key]
    res = run_bass_kernel_spmd(nc, in_maps, core_ids=list(range(N_CORES)),
                               trace=trace, **spmd_kwargs)
    return gather(res.results), res


def kernel(**inputs) -> np.ndarray:
    y, _ = _run(inputs, trace=False)
    return y


# revision 18
# speedup vs baseline: 1.4121x; 1.1335x over previous
"""TRN2 Bass kernel for nn_Attention_5720896438407 (8-core data-parallel).

Mathematical collapse: the module computes SDPA over the *head* axis with a
single KV head (KV=1), so every attention weight is exactly 1.0 and the whole
module reduces to (see kernel_baseline.py.bak for the derivation)

    T  = hidden @ kv_a_w.T + kv_a_b                    # (ntok, 512)
    s  = rsqrt(mean(T^2, -1) + eps)                    # per-token RMS scale
    V  = (s*T) @ Wv' + bv,   Wv' = (kv_b_w[128:256] * (1+kv_norm_w)).T
    Y  = V @ M.T,            M   = o_w.reshape(2048, 16, 128).sum(1)

This version additionally *folds the value path past the RMS norm*: since s
is a per-token scalar,

    V = s * (hidden @ Wf) + bv,   Wf = kv_a_w.T @ Wv'   (2048 x 128)

so T is only needed for the statistic s.  That lets the dominant matmul
(hidden @ kv_a_w.T, 2048-dim contraction, 512 outputs) run in fp8 DoubleRow
mode (2 MACs/cell/cycle): fp8 quantization errors are independent across the
512 columns, so the *mean* of T^2 — and hence s — keeps ~0.2% accuracy while
the matmul runs at 2x rate.  The accurate value path is the cheap rank-128
fold (hidden @ Wf, fp16) plus the output projection (V @ M.T, fp16).
Numerically verified: ~2e-3 rel-fro error vs the fp32 reference.

Distribution: pure data-parallel over the 8192 tokens — 1024 per core, no
collectives.  Per core, tokens stream in 8 slabs of 128 (two 512-token
super-slabs); the fp8 copy of X is cast on-chip by the (otherwise idle)
GpSimd engine, except the first 3 slabs which ship pre-cast from the host so
the PE can start at ~2us.  Engine layout per slab: PE does 8 DoubleRow MMs
(statistic) + 4 fp16 MMs (value, batched over the super-slab) + 4 fp16 MMs
(output); ACT does the Square+accum statistic and half the PSUM->SBUF
copies; DVE does s=1/sqrt and the other half of the copies, applying the
per-token s during the copy.  Input X/weights and output Y ride different
HWDGE rings (SP / ACT) and fully overlap compute.
"""
import sys

sys.path.insert(0, "/opt/trn_rl_repo")

import numpy as np
import ml_dtypes
import concourse.bass as bass
import concourse.tile as tile
from concourse import bacc, mybir
from concourse.bass_utils import run_bass_kernel_spmd
from concourse.masks import make_identity

F32 = mybir.dt.float32
F16 = mybir.dt.float16
F8 = mybir.dt.float8e4
DRMODE = mybir.MatmulPerfMode.DoubleRow
AF = mybir.ActivationFunctionType

HID = 2048
KV = 512
D = 128
OUT = 2048
EPS = 1e-6
WSC = 64.0                    # fp8 scale on kv_a_w (entries ~0.02)
SQ_SCALE = 1.0 / (KV * WSC * WSC)
SLAB = 128                    # tokens per slab
SS_TOK = 512                  # tokens per super-slab (value-matmul batch)
NPRE = 3                      # X8 slabs pre-cast on host
N_CORES = 8
E4 = ml_dtypes.float8_e4m3

_NC_CACHE = {}


def _build_nc(tok, with_ba):
    nss = tok // SS_TOK
    assert tok % SS_TOK == 0 and nss == 2

    nc = bacc.Bacc("TRN2", target_bir_lowering=False, debug=False,
                   num_devices=1)

    x16_d = nc.dram_tensor("x16", (nss, 128, 4, 16, SLAB), F16,
                           kind="ExternalInput").ap()
    x8p_d = nc.dram_tensor("x8p", (nss, 128, 4, 8, 2, SLAB), F8,
                           kind="ExternalInput").ap()
    w18_d = nc.dram_tensor("w18", (128, 8, 2, KV), F8,
                           kind="ExternalInput").ap()
    wf_d = nc.dram_tensor("wf", (128, 16, D), F16, kind="ExternalInput").ap()
    mt_d = nc.dram_tensor("mt", (D, OUT), F16, kind="ExternalInput").ap()
    if with_ba:
        bar8_d = nc.dram_tensor("bar8", (1, KV), F8,
                                kind="ExternalInput").ap()
        cvt_d = nc.dram_tensor("cvt", (1, D), F16, kind="ExternalInput").ap()
    y_d = nc.dram_tensor("y", (tok, OUT), F16, kind="ExternalOutput").ap()

    with tile.TileContext(nc) as tc:
        with tc.tile_pool(name="consts", bufs=1) as consts, \
             tc.tile_pool(name="xs16", bufs=2) as xs16, \
             tc.tile_pool(name="xs8", bufs=2) as xs8, \
             tc.tile_pool(name="work", bufs=2) as work, \
             tc.tile_pool(name="ps_t", bufs=2, space="PSUM") as ps_t, \
             tc.tile_pool(name="ps_v", bufs=2, space="PSUM") as ps_v, \
             tc.tile_pool(name="ps_y", bufs=3, space="PSUM") as ps_y:
            # ---- DMA schedule.  Ring rates are ~210 GB/s each (~0.21
            #      MB/us); SP's first packet lands ~1.5 us before ACT's.
            #      First bytes on each ring are the statistic-path operands
            #      (w18 quarters + x8 slab 0) so the PE can start at ~11 us;
            #      the X16 stream follows on SP; ACT carries the remaining
            #      x8 slabs and the small weights. ----
            w18_s = consts.tile([128, 8, 2, KV], F8, tag="w18")
            x16_t = [xs16.tile([128, 4, 16, SLAB], F16, tag="x16",
                               name=f"x16_{ss}") for ss in range(nss)]
            x8_t = [xs8.tile([128, 4, 8, 2, SLAB], F8, tag="x8",
                             name=f"x8_{ss}") for ss in range(nss)]
            # SP ring (consumption order: statistic head, then X16 stream)
            nc.sync.dma_start(x8_t[0][:, 0], x8p_d[0, :, 0])
            nc.sync.dma_start(w18_s[:, 0:2], w18_d[:, 0:2])
            for j in range(4):
                nc.sync.dma_start(x16_t[0][:, j], x16_d[0, :, j])
            for j in range(4):
                nc.sync.dma_start(x16_t[1][:, j], x16_d[1, :, j])
            # ACT ring (w18 quarters, early x8 slabs, weights, x8 ss1)
            wf_s = consts.tile([128, 16, D], F16, tag="wf")
            mt_s = consts.tile([D, OUT], F16, tag="mt")
            for p in range(2, 8, 2):
                nc.scalar.dma_start(w18_s[:, p:p + 2], w18_d[:, p:p + 2])
            nc.scalar.dma_start(x8_t[0][:, 1], x8p_d[0, :, 1])
            nc.scalar.dma_start(x8_t[0][:, 2], x8p_d[0, :, 2])
            nc.scalar.dma_start(wf_s[:], wf_d)
            nc.scalar.dma_start(mt_s[:], mt_d)
            nc.scalar.dma_start(x8_t[0][:, 3], x8p_d[0, :, 3])
            for j in range(4):
                nc.scalar.dma_start(x8_t[1][:, j], x8p_d[1, :, j])
            if with_ba:
                bar8_s = consts.tile([1, KV], F8, tag="bar8")
                nc.scalar.dma_start(bar8_s[:], bar8_d)
                cvt_s = consts.tile([1, D], F16, tag="cvt")
                nc.scalar.dma_start(cvt_s[:], cvt_d)
                ones8_s = consts.tile([1, 128], F8, tag="ones8")
                nc.vector.memset(ones8_s[:], 1.0)
                ones16_s = consts.tile([1, SS_TOK], F16, tag="ones16")
                nc.vector.memset(ones16_s[:], 1.0)
            eps_s = consts.tile([128, 1], F32, tag="eps")
            nc.vector.memset(eps_s[:], EPS)

            # ---- PE warm-up / keep-alive: junk matmuls on an early-ready
            #      zero tile keep the HAM activity monitor from throttling
            #      the PE while data DMAs are in flight ----
            js = consts.tile([128, 512], F16, tag="js")
            nc.gpsimd.memset(js[:], 0.0)
            junkt = [ps_y.tile([128, 1024], F32, tag="py", bufs=2, name=f"junk{i}")
                     for i in range(2)]
            jn = [0]

            def junk(n):
                for _ in range(n):
                    i = jn[0] = jn[0] + 1
                    nc.tensor.matmul(junkt[i % 2][:, 0:512], js[:, 0:128],
                                     js[:], start=True, stop=True)

            junk(8)

            def norm_mm(ss, j, filler=False):
                # statistic matmul: T = X @ W1 in fp8 DoubleRow, token-major
                pt = ps_t.tile([128, KV], F32, tag="pt", name=f"pt{ss}_{j}")
                for p in range(8):
                    nc.tensor.matmul(pt[:], x8_t[ss][:, j, p], w18_s[:, p],
                                     start=(p == 0),
                                     stop=(p == 7 and not with_ba),
                                     perf_mode=DRMODE)
                    if filler and p % 2 == 1 and p < 7:
                        # first slab is paced by the w18 quarter DMAs; keep
                        # the PE clock warm between pair arrivals
                        junk(1)
                if with_ba:
                    # rank-1 row-broadcast of 64*kv_a_b into the accumulation
                    nc.tensor.matmul(pt[:], ones8_s[:], bar8_s[:],
                                     start=False, stop=True)
                return pt

            def stats(ss, j, pt):
                sq = work.tile([128, KV], F8, tag="sq", bufs=2)
                ssq = work.tile([128, 1], F32, tag="ssq", bufs=2)
                nc.scalar.activation(sq[:], pt[:], AF.Square,
                                     accum_out=ssq[:])
                rt = work.tile([128, 1], F32, tag="rt", bufs=2)
                nc.scalar.activation(rt[:], ssq[:], AF.Sqrt, bias=eps_s[:],
                                     scale=SQ_SCALE)
                sc = work.tile([128, 1], F32, tag="sc", bufs=8,
                               name=f"sc{ss}_{j}")
                nc.vector.reciprocal(sc[:], rt[:])
                return sc

            def value_mm(ss, h):
                # V.T = Wf.T @ X.T for one half-super-slab (2 slabs,
                # N=256), d-major; halves unblock as their X16 slabs land
                pv = ps_v.tile([128, 2 * SLAB], F32, tag="pv", bufs=2,
                               name=f"pv{ss}_{h}")
                for ck in range(16):
                    nc.tensor.matmul(pv[:], wf_s[:, ck],
                                     x16_t[ss][:, 2 * h:2 * h + 2, ck, :],
                                     start=(ck == 0),
                                     stop=(ck == 15 and not with_ba))
                if with_ba:
                    # rank-1: + (kv_a_b @ Wv') per-d constant over tokens
                    nc.tensor.matmul(pv[:], cvt_s[:],
                                     ones16_s[:, 0:2 * SLAB],
                                     start=False, stop=True)
                vts = work.tile([128, 2 * SLAB], F16, tag="vts", bufs=4,
                                name=f"vts{ss}_{h}")
                nc.scalar.activation(vts[:], pv[:], AF.Copy, bias=0.0,
                                     scale=1.0)
                return vts

            def step4(ss, j, vts, sc, last):
                # Y slab = s * (V.T-slab.T @ M.T); s applied during the
                # PSUM->SBUF copy (per-partition scalar on token-major out)
                t0 = (ss * 4 + j) * SLAB
                jh = (j % 2) * SLAB
                ysb = work.tile([128, OUT], F16, tag="ysb", bufs=4,
                                name=f"ysb{ss}_{j}")
                for h in range(2):
                    # two matmuls into one 2-bank PSUM tile, then a single
                    # wide scaled copy (DVE first half, ACT second half) so
                    # the copies never pace the matmuls
                    py = ps_y.tile([128, 1024], F32, tag="py", bufs=2,
                                   name=f"py{ss}_{j}_{h}")
                    for n in range(2):
                        nc.tensor.matmul(py[:, n * 512:(n + 1) * 512],
                                         vts[:, jh:jh + SLAB],
                                         mt_s[:, (2 * h + n) * 512:
                                              (2 * h + n + 1) * 512],
                                         start=True, stop=True)
                    ysl = ysb[:, h * 1024:(h + 1) * 1024]
                    if h == 0:
                        nc.vector.tensor_scalar_mul(ysl, py[:], sc[:])
                    else:
                        nc.scalar.activation(ysl, py[:], AF.Copy, bias=0.0,
                                             scale=sc[:])
                    if last and h == 0:
                        # final slab: overlap the first output half with the
                        # remaining matmuls/copies so only 0.25 MB trails
                        nc.scalar.dma_start(y_d[t0:t0 + SLAB, 0:1024],
                                            ysb[:, 0:1024])
                # alternate output DMAs across the two HWDGE rings
                eng = nc.scalar if (ss * 4 + j) % 2 == 0 else nc.sync
                if last:
                    nc.scalar.dma_start(y_d[t0:t0 + SLAB, 1024:2048],
                                        ysb[:, 1024:2048])
                else:
                    eng.dma_start(y_d[t0:t0 + SLAB, :], ysb[:])

            # ---- pipeline emission (PE program order == expected readiness
            #      order so the FIFO never head-of-line blocks) ----
            scs = {}
            scs[(0, 0)] = stats(0, 0, norm_mm(0, 0, filler=True))
            junk(2)
            scs[(0, 1)] = stats(0, 1, norm_mm(0, 1))
            scs[(0, 2)] = stats(0, 2, norm_mm(0, 2))
            v0a = value_mm(0, 0)
            scs[(0, 3)] = stats(0, 3, norm_mm(0, 3))
            v0b = value_mm(0, 1)
            step4(0, 0, v0a, scs[(0, 0)], False)
            step4(0, 1, v0a, scs[(0, 1)], False)
            scs[(1, 0)] = stats(1, 0, norm_mm(1, 0))
            step4(0, 2, v0b, scs[(0, 2)], False)
            step4(0, 3, v0b, scs[(0, 3)], False)
            scs[(1, 1)] = stats(1, 1, norm_mm(1, 1))
            scs[(1, 2)] = stats(1, 2, norm_mm(1, 2))
            v1a = value_mm(1, 0)
            scs[(1, 3)] = stats(1, 3, norm_mm(1, 3))
            v1b = value_mm(1, 1)
            step4(1, 0, v1a, scs[(1, 0)], False)
            step4(1, 1, v1a, scs[(1, 1)], False)
            step4(1, 2, v1b, scs[(1, 2)], False)
            step4(1, 3, v1b, scs[(1, 3)], True)

    nc.compile()
    return nc


def _host_prep(inputs):
    """Fold weights, swizzle X into fp16 slab layout, shard across cores."""
    h = np.asarray(inputs["hidden_states"], dtype=np.float32)
    b, s, hid = h.shape
    assert hid == HID
    x = np.ascontiguousarray(h.reshape(b * s, hid))
    ntok = b * s
    tok = ntok // N_CORES
    nss = tok // SS_TOK

    kv_a_w = np.asarray(inputs["kv_a_w"], np.float32)
    kv_a_b = np.asarray(inputs["kv_a_b"], np.float32)
    kv_norm_w = np.asarray(inputs["kv_norm_w"], np.float32)
    kv_b_w = np.asarray(inputs["kv_b_w"], np.float32)
    kv_b_b = np.asarray(inputs["kv_b_b"], np.float32)
    o_w = np.asarray(inputs["o_w"], np.float32)

    W1 = np.ascontiguousarray(kv_a_w.T)                       # (2048, 512)
    Wvp = np.ascontiguousarray(
        (kv_b_w[D:2 * D] * (1.0 + kv_norm_w)[None, :]).T)     # (512, 128)
    Wf = W1 @ Wvp                                             # (2048, 128)
    Mh = o_w.reshape(HID, 16, D).sum(axis=1)                  # (2048, 128)

    w18 = np.clip(W1 * WSC, -240, 240).reshape(16, 128, KV) \
        .transpose(1, 0, 2).reshape(128, 8, 2, KV).astype(E4)
    wfh = np.ascontiguousarray(
        Wf.reshape(16, 128, D).transpose(1, 0, 2)).astype(np.float16)
    mth = np.ascontiguousarray(Mh.T).astype(np.float16)       # (128, 2048)

    with_ba = bool(np.any(kv_a_b != 0.0))
    consts = {"w18": w18, "wf": wfh, "mt": mth}
    if with_ba:
        consts["bar8"] = np.clip(kv_a_b * WSC, -240, 240) \
            .reshape(1, KV).astype(E4)
        consts["cvt"] = (kv_a_b @ Wvp).reshape(1, D).astype(np.float16)

    in_maps = []
    for i in range(N_CORES):
        shard = x[i * tok:(i + 1) * tok]
        x16 = np.ascontiguousarray(
            shard.reshape(nss, 4, SLAB, 16, 128).transpose(0, 4, 1, 3, 2)
        ).astype(np.float16)
        x8p = np.clip(x16.astype(np.float32), -240, 240) \
            .reshape(nss, 128, 4, 8, 2, SLAB).astype(E4)
        m = dict(consts)
        m["x16"] = x16
        m["x8p"] = x8p
        in_maps.append(m)

    bvrow = None
    if np.any(kv_b_b[D:2 * D] != 0.0):
        bvrow = (kv_b_b[D:2 * D] @ Mh.T).astype(np.float32)   # (2048,)

    def gather(results):
        y = np.concatenate([r["y"] for r in results], axis=0) \
            .astype(np.float32)
        if bvrow is not None:
            y += bvrow[None, :]
        return np.ascontiguousarray(y.reshape(b, s, HID))

    return in_maps, gather, with_ba, tok


def _run(inputs, trace=False, **spmd_kwargs):
    in_maps, gather, with_ba, tok = _host_prep(inputs)
    key = (tok, with_ba)
    if key not in _NC_CACHE:
        _NC_CACHE[key] = _build_nc(tok, with_ba)
    nc = _NC_CACHE[key]
    res = run_bass_kernel_spmd(nc, in_maps, core_ids=list(range(N_CORES)),
                               trace=trace, **spmd_kwargs)
    return gather(res.results), res


def kernel(**inputs) -> np.ndarray:
    y, _ = _run(inputs, trace=False)
    return y
